# revision 10
# baseline (speedup 1.0000x reference)
"""Conv2D-KAN Trainium2 kernel (8-core data-parallel SPMD), v3.

Formulation (per 3x3 patch, N = B*30*30 patches, in_size = 288):
    out[n,o] = sum_{i,k} sb[n,i,k] * (spline_kernel*scale)[i,k,o]
             + silu(xf) @ scale_factor + biases
with sb a cubic B-spline basis (8 funcs) over uniform knots
t_r = -2.2 + 0.4 r (r = 0..11, h = 0.4).

Device pipeline per image (4 images per core):
 1. x replicated to [128 = 4r x 32c, 1024 pix] (one DMA set).
 2. T_r = relu((x - t_r)/h)^3 in fp32 (relu+square on ACT, cube-mul
    on DVE), 3 tiles of [4r x 32c, 1024]. No clamp: the 4th-difference
    identity sum_m cm_m T_{k+m} = 6 B_k holds unclamped everywhere
    (it vanishes identically right of each basis support).
 3. The blend needs fp32 T (large cubes cancel), but fp32 matmuls run
    at 4 cyc/row. T is split hi/lo: hi = bf16(T) (exact in PE bf16
    mode), lo = T - hi (exact Sterbenz difference; |lo| <= ulp so
    tf32 rounding of it is harmless). Blend = banded matrices Ma/Mb
    (cm = [1,-4,6,-4,1], /6 folded into weights) against hi (bf16)
    and lo (f32r), 1 cyc/row, stationary-major over 4 PSUM banks so
    each matrix loads into the PE once per image -> B tiles (bf16).
 4. silu: SL3 [96 = 3dj x 32c, 1024] holds silu(x) column-shifted by
    dj so the 9 silu conv offsets collapse into 3 matmuls of K=96.
 5. Main conv, weight-stationary: 21 chunks (3 silu K=96 + 18 basis
    K=128), each loaded once and streamed into both PSUM banks
    [128 filters, 450 patches] of the image, all bf16 at 1 cyc/row,
    then bias via ACT and DMA out.

Output [128, 3600] per core is transposed on host.
"""

import sys

sys.path.insert(0, "/opt/trn_rl_repo")

import numpy as np

N_CORES = 8
B, HH, WW, C = 32, 32, 32, 32
F = 128
KH = KW = 3
HO, WO = HH - KH + 1, WW - KW + 1          # 30, 30
BPC = B // N_CORES                          # images per core = 4
PIX = HH * WW                               # 1024 pixels per image
NPC = BPC * HO * WO                         # 3600 patches per core
BANKN = 450                                 # patches per psum bank
HGRID = 0.4
T0 = -2.2                                   # first knot
NR = 11

_cache = {}


def _build_program():
    import concourse.bacc as bacc
    import concourse.mybir as mybir
    import concourse.tile as tile

    f32 = mybir.dt.float32
    f32r = mybir.dt.float32r
    bf16 = mybir.dt.bfloat16
    f16 = mybir.dt.float16
    AF = mybir.ActivationFunctionType

    nc = bacc.Bacc("TRN2", target_bir_lowering=False, debug=False)
    xt = nc.dram_tensor("xt", [C, BPC * PIX], f32, kind="ExternalInput").ap()
    # basis chunk weights, chunk = g*9+off, rows k-major (4k x 32c)
    wb = nc.dram_tensor("wb", [128, 18 * F], bf16, kind="ExternalInput").ap()
    # silu chunk weights, chunk = di, rows 3dj x 32c
    ws = nc.dram_tensor("ws", [96, 3 * F], bf16, kind="ExternalInput").ap()
    # banded blend matrices Ma|Mb (cm values)
    wm = nc.dram_tensor("wm", [128, 2 * 128], f32, kind="ExternalInput").ap()
    consts = nc.dram_tensor("consts", [128, 8], f32, kind="ExternalInput").ap()
    y = nc.dram_tensor("y", [F, NPC], f32, kind="ExternalOutput").ap()

    with tile.TileContext(nc) as tc:
        with (
            tc.tile_pool(name="wp", bufs=1) as wp,
            tc.tile_pool(name="xp", bufs=2) as xp,
            tc.tile_pool(name="tp", bufs=2) as tp,
            tc.tile_pool(name="hp", bufs=2) as hp,
            tc.tile_pool(name="lp", bufs=2) as lp,
            tc.tile_pool(name="bp", bufs=2) as bp,
            tc.tile_pool(name="op", bufs=1) as op_,
            tc.tile_pool(name="pp", bufs=2, space="PSUM") as pp,
            tc.tile_pool(name="pb", bufs=1, space="PSUM") as pb,
        ):
            ct = wp.tile([128, 8], f32)
            nc.scalar.dma_start(ct[:], consts[:])

            # warm the ACT table set (silu's set carries relu / copy /
            # identity / square) before the first feature tile lands.
            warm = wp.tile([1, 1], f32, tag="warm")
            nc.scalar.activation(warm[:], ct[:1, :1], AF.Silu)

            # image 0's x-replica: four DMAs split across two queues
            # ahead of all other traffic.
            xr0 = xp.tile([128, PIX], f32, tag="xr")
            eng = [nc.sync, nc.gpsimd, nc.sync, nc.gpsimd]
            for rep in range(4):
                eng[rep].dma_start(xr0[32 * rep:32 * rep + 32], xt[:, 0:PIX])

            # weights (scalar queue: idle until image 0 features exist)
            wbt = wp.tile([128, 18 * F], bf16, tag="wb")
            nc.scalar.dma_start(wbt[:], wb[:])
            wst = wp.tile([96, 3 * F], bf16, tag="ws")
            nc.scalar.dma_start(wst[:], ws[:])
            wmt = wp.tile([128, 2 * 128], f32, tag="wm")
            nc.scalar.dma_start(wmt[:], wm[:])
            # blend matrices in fp16 (hi and lo both fp16, 1 cyc/row)
            mF = wp.tile([128, 2 * 128], f16, tag="mF")
            nc.vector.tensor_copy(mF[:], wmt[:])
            stats = [mF[:, 0:128], mF[:, 128:256]]         # Ma, Mb

            out_t = op_.tile([F, NPC], f32)

            for im in range(BPC):
                sl = slice(im * PIX, (im + 1) * PIX)
                if im == 0:
                    xr = xr0
                else:
                    xr = xp.tile([128, PIX], f32, tag="xr")
                    for rep in range(4):
                        eng[rep].dma_start(
                            xr[32 * rep:32 * rep + 32], xt[:, sl])

                his, los = [], []

                def build_t(t):
                    T = tp.tile([128, PIX], f32, tag=f"T{t}", name=f"T{t}")
                    nc.scalar.activation(
                        T[:], xr[:], AF.Relu,
                        bias=ct[:, t:t + 1], scale=1.0 / HGRID)
                    sq = tp.tile([128, PIX], f32, tag="sq", name="sq")
                    nc.scalar.activation(sq[:], T[:], AF.Square)
                    nc.vector.tensor_mul(T[:], sq[:], T[:])
                    hi = hp.tile([128, PIX], f16, tag=f"h{t}", name=f"h{t}")
                    nc.vector.tensor_copy(hi[:], T[:])
                    lo = lp.tile([128, PIX], f16, tag=f"l{t}", name=f"l{t}")
                    nc.gpsimd.tensor_sub(lo[:], T[:], hi[:])
                    his.append(hi)
                    los.append(lo)

                def blend_group(g):
                    # per-group: Ma/Mb load once each (2 LDW), B_g copies
                    # issue on DVE before the next T chain queues there.
                    pbt = [pb.tile([128, 512], f32, tag=f"pb{g}{hf}",
                                   name=f"pb{g}{hf}") for hf in range(2)]
                    for si, (stat, srcs, d) in enumerate((
                            (stats[0], his, 0), (stats[0], los, 0),
                            (stats[1], his, 1), (stats[1], los, 1))):
                        for hf in range(2):
                            hs = slice(hf * 512, (hf + 1) * 512)
                            nc.tensor.matmul(
                                pbt[hf][:], stat, srcs[g + d][:, hs],
                                start=(si == 0), stop=(si == 3))
                    Bt = bp.tile([128, PIX], bf16, tag=f"B{g}",
                                 name=f"B{g}")
                    for hf in range(2):
                        hs = slice(hf * 512, (hf + 1) * 512)
                        nc.vector.tensor_copy(Bt[:, hs], pbt[hf][:])
                    return Bt[:].rearrange("p (h w) -> p h w", w=WW)

                # t2 and silu are deferred past group-0's blend so B0 (and
                # with it the g0 main chunks) is ready as early as possible.
                build_t(0)
                build_t(1)
                Bviews = [blend_group(0)]

                sl3 = xp.tile([96, PIX], bf16, tag="sl3")
                for g in range(3):
                    nc.scalar.activation(
                        sl3[32 * g:32 * g + 32, 0:PIX - g],
                        xr[0:32, g:PIX], AF.Silu)
                slv = sl3[:].rearrange("p (h w) -> p h w", w=WW)

                build_t(2)
                Bviews.append(blend_group(1))

                # --- main conv, weight-stationary over both banks ---
                pss = [pp.tile([F, BANKN], f32, tag=f"ps{hf}",
                               name=f"ps{hf}")
                       for hf in range(2)]
                for ci in range(21):
                    if ci < 3:
                        di = ci
                        lhsT = wst[:, di * F:(di + 1) * F]
                        rhs = [slv[:, half * 15 + di:half * 15 + di + 15, 0:WO]
                               for half in range(2)]
                    else:
                        g, off = divmod(ci - 3, 9)
                        di, dj = divmod(off, KW)
                        lhsT = wbt[:, (g * 9 + off) * F:(g * 9 + off + 1) * F]
                        rhs = [Bviews[g][:, half * 15 + di:half * 15 + di + 15,
                                         dj:dj + WO]
                               for half in range(2)]
                    for half in range(2):
                        nc.tensor.matmul(
                            pss[half][:], lhsT, rhs[half],
                            start=(ci == 0), stop=(ci == 20))
                for half in range(2):
                    s = (im * 2 + half) * BANKN
                    nc.scalar.activation(
                        out_t[:, s:s + BANKN], pss[half][:], AF.Identity,
                        bias=ct[:, 6:7], scale=1.0)
                    nc.sync.dma_start(y[:, s:s + BANKN], out_t[:, s:s + BANKN])

    nc.compile()
    return nc


def _prep_static(spline_kernel, scale_factor, kan_bias, conv_bias):
    import ml_dtypes

    w6 = (spline_kernel.astype(np.float64)
          * scale_factor.astype(np.float64)[:, None, :]) / 6.0
    w6 = w6.reshape(KH * KW, C, 8, F)
    Wb = np.zeros((18, 128, F), np.float64)
    for off in range(9):
        for g in range(2):
            blk = w6[off][:, 4 * g:4 * g + 4]            # (32c, 4k, F)
            Wb[g * 9 + off] = blk.transpose(1, 0, 2).reshape(128, F)
    wb = np.ascontiguousarray(
        Wb.transpose(1, 0, 2).reshape(128, 18 * F)).astype(ml_dtypes.bfloat16)

    sf9 = scale_factor.astype(np.float64).reshape(9, C, F)
    Ws = np.zeros((3, 96, F), np.float64)
    for di in range(3):
        Ws[di] = sf9[3 * di:3 * di + 3].reshape(96, F)
    ws = np.ascontiguousarray(
        Ws.transpose(1, 0, 2).reshape(96, 3 * F)).astype(ml_dtypes.bfloat16)

    cm = np.array([1.0, -4.0, 6.0, -4.0, 1.0])
    pin = np.arange(128)[:, None]
    pout = np.arange(128)[None, :]
    same_c = (pin % 32) == (pout % 32)
    Ms = []
    for base in (0, 4):
        m = base + pin // 32 - pout // 32
        Ms.append(np.where((m >= 0) & (m <= 4) & same_c,
                           cm[np.clip(m, 0, 4)], 0.0))
    wm = np.ascontiguousarray(
        np.concatenate(Ms, axis=1), np.float32)          # [128, 256]

    consts = np.zeros((128, 8), np.float32)
    p = np.arange(128)
    for t in range(3):
        r = 4 * t + p // 32
        consts[:, t] = -(T0 + HGRID * r) / HGRID          # 5.5 - r
    consts[:, 6] = (kan_bias.astype(np.float64)
                    + conv_bias.astype(np.float64)).astype(np.float32)
    return wb, ws, wm, consts


def kernel(x, spline_kernel, scale_factor, kan_bias, conv_bias):
    from concourse import bass_utils

    x = np.asarray(x, np.float32)
    spline_kernel = np.asarray(spline_kernel, np.float32)
    scale_factor = np.asarray(scale_factor, np.float32)
    kan_bias = np.asarray(kan_bias, np.float32)
    conv_bias = np.asarray(conv_bias, np.float32)

    if "nc" not in _cache:
        _cache["nc"] = _build_program()
    nc = _cache["nc"]

    wb, ws, wm, consts = _prep_static(
        spline_kernel, scale_factor, kan_bias, conv_bias)

    in_maps = []
    for c in range(N_CORES):
        xc = x[c * BPC:(c + 1) * BPC]                     # (4,32,32,32)
        xtc = np.ascontiguousarray(
            xc.transpose(3, 0, 1, 2).reshape(C, BPC * PIX), np.float32
        )
        in_maps.append(
            {"xt": xtc, "wb": wb, "ws": ws, "wm": wm, "consts": consts})

    res = bass_utils.run_bass_kernel_spmd(
        nc, in_maps, core_ids=list(range(N_CORES)),
        **_cache.get("run_kwargs", {})
    )
    _cache["last_result"] = res

    out = np.empty((B, HO, WO, F), np.float32)
    for c in range(N_CORES):
        yc = res.results[c]["y"]                          # (128, 3600)
        out[c * BPC:(c + 1) * BPC] = (
            yc.reshape(F, BPC, HO, WO).transpose(1, 2, 3, 0)
        )
    return out


# revision 11
# speedup vs baseline: 1.1530x; 1.1530x over previous
"""Conv2D-KAN Trainium2 kernel (8-core data-parallel SPMD), v3.

Formulation (per 3x3 patch, N = B*30*30 patches, in_size = 288):
    out[n,o] = sum_{i,k} sb[n,i,k] * (spline_kernel*scale)[i,k,o]
             + silu(xf) @ scale_factor + biases
with sb a cubic B-spline basis (8 funcs) over uniform knots
t_r = -2.2 + 0.4 r (r = 0..11, h = 0.4).

Device pipeline per image (4 images per core):
 1. x replicated to [128 = 4r x 32c, 1024 pix] (one DMA set).
 2. T_r = relu((x - t_r)/h)^3 in fp32 (relu+square on ACT, cube-mul
    on DVE), 3 tiles of [4r x 32c, 1024]. No clamp: the 4th-difference
    identity sum_m cm_m T_{k+m} = 6 B_k holds unclamped everywhere
    (it vanishes identically right of each basis support).
 3. The blend needs fp32 T (large cubes cancel), but fp32 matmuls run
    at 4 cyc/row. T is split hi/lo: hi = bf16(T) (exact in PE bf16
    mode), lo = T - hi (exact Sterbenz difference; |lo| <= ulp so
    tf32 rounding of it is harmless). Blend = banded matrices Ma/Mb
    (cm = [1,-4,6,-4,1], /6 folded into weights) against hi (bf16)
    and lo (f32r), 1 cyc/row, stationary-major over 4 PSUM banks so
    each matrix loads into the PE once per image -> B tiles (bf16).
 4. silu: SL3 [96 = 3dj x 32c, 1024] holds silu(x) column-shifted by
    dj so the 9 silu conv offsets collapse into 3 matmuls of K=96.
 5. Main conv, weight-stationary: 21 chunks (3 silu K=96 + 18 basis
    K=128), each loaded once and streamed into both PSUM banks
    [128 filters, 450 patches] of the image, all bf16 at 1 cyc/row,
    then bias via ACT and DMA out.

Output [128, 3600] per core is transposed on host.
"""

import sys

sys.path.insert(0, "/opt/trn_rl_repo")

import numpy as np

N_CORES = 8
B, HH, WW, C = 32, 32, 32, 32
F = 128
KH = KW = 3
HO, WO = HH - KH + 1, WW - KW + 1          # 30, 30
BPC = B // N_CORES                          # images per core = 4
PIX = HH * WW                               # 1024 pixels per image
NPC = BPC * HO * WO                         # 3600 patches per core
BANKN = 450                                 # patches per psum bank
HGRID = 0.4
T0 = -2.2                                   # first knot
NR = 11

_cache = {}


def _build_program():
    import concourse.bacc as bacc
    import concourse.mybir as mybir
    import concourse.tile as tile

    f32 = mybir.dt.float32
    f32r = mybir.dt.float32r
    bf16 = mybir.dt.bfloat16
    f16 = mybir.dt.float16
    AF = mybir.ActivationFunctionType

    nc = bacc.Bacc("TRN2", target_bir_lowering=False, debug=False)
    xt = nc.dram_tensor("xt", [C, BPC * PIX], f32, kind="ExternalInput").ap()
    # basis chunk weights, chunk = g*9+off, rows k-major (4k x 32c)
    wb = nc.dram_tensor("wb", [128, 18 * F], bf16, kind="ExternalInput").ap()
    # silu chunk weights, chunk = di, rows 3dj x 32c
    ws = nc.dram_tensor("ws", [96, 3 * F], bf16, kind="ExternalInput").ap()
    # banded blend matrices Ma|Mb (cm values)
    wm = nc.dram_tensor("wm", [128, 2 * 128], f32, kind="ExternalInput").ap()
    consts = nc.dram_tensor("consts", [128, 8], f32, kind="ExternalInput").ap()
    y = nc.dram_tensor("y", [F, NPC], f32, kind="ExternalOutput").ap()

    with tile.TileContext(nc) as tc:
        with (
            tc.tile_pool(name="wp", bufs=1) as wp,
            tc.tile_pool(name="xp", bufs=2) as xp,
            tc.tile_pool(name="tp", bufs=2) as tp,
            tc.tile_pool(name="hp", bufs=2) as hp,
            tc.tile_pool(name="lp", bufs=2) as lp,
            tc.tile_pool(name="bp", bufs=2) as bp,
            tc.tile_pool(name="op", bufs=1) as op_,
            tc.tile_pool(name="pp", bufs=2, space="PSUM") as pp,
            tc.tile_pool(name="pb", bufs=1, space="PSUM") as pb,
        ):
            ct = wp.tile([128, 8], f32)
            nc.scalar.dma_start(ct[:], consts[:])

            # warm the ACT table set (silu's set carries relu / copy /
            # identity / square) before the first feature tile lands.
            warm = wp.tile([1, 1], f32, tag="warm")
            nc.scalar.activation(warm[:], ct[:1, :1], AF.Silu)

            # image 0's x-replica: four DMAs split across two queues
            # ahead of all other traffic.
            xr0 = xp.tile([128, PIX], f32, tag="xr")
            eng = [nc.sync, nc.gpsimd, nc.sync, nc.gpsimd]
            for rep in range(4):
                eng[rep].dma_start(xr0[32 * rep:32 * rep + 32], xt[:, 0:PIX])

            # weights (scalar queue: idle until image 0 features exist)
            wbt = wp.tile([128, 18 * F], bf16, tag="wb")
            nc.scalar.dma_start(wbt[:], wb[:])
            wst = wp.tile([96, 3 * F], bf16, tag="ws")
            nc.scalar.dma_start(wst[:], ws[:])
            wmt = wp.tile([128, 2 * 128], f32, tag="wm")
            nc.scalar.dma_start(wmt[:], wm[:])
            # blend matrices in fp16 (hi and lo both fp16, 1 cyc/row)
            mF = wp.tile([128, 2 * 128], f16, tag="mF")
            nc.vector.tensor_copy(mF[:], wmt[:])
            stats = [mF[:, 0:128], mF[:, 128:256]]         # Ma, Mb

            out_t = op_.tile([F, NPC], f32)

            for im in range(BPC):
                sl = slice(im * PIX, (im + 1) * PIX)
                if im == 0:
                    xr = xr0
                else:
                    xr = xp.tile([128, PIX], f32, tag="xr")
                    for rep in range(4):
                        eng[rep].dma_start(
                            xr[32 * rep:32 * rep + 32], xt[:, sl])

                # --- silu tile, dj-shifted into partition groups ---
                sl3 = xp.tile([96, PIX], bf16, tag="sl3")
                for g in range(3):
                    nc.scalar.activation(
                        sl3[32 * g:32 * g + 32, 0:PIX - g],
                        xr[0:32, g:PIX], AF.Silu)
                slv = sl3[:].rearrange("p (h w) -> p h w", w=WW)

                # --- T tiles + hi/lo split ---
                his, los = [], []
                for t in range(3):
                    T = tp.tile([128, PIX], f32, tag=f"T{t}")
                    nc.scalar.activation(
                        T[:], xr[:], AF.Relu,
                        bias=ct[:, t:t + 1], scale=1.0 / HGRID)
                    sq = tp.tile([128, PIX], f32, tag="sq")
                    nc.scalar.activation(sq[:], T[:], AF.Square)
                    nc.vector.tensor_mul(T[:], sq[:], T[:])
                    hi = hp.tile([128, PIX], f16, tag=f"h{t}")
                    nc.vector.tensor_copy(hi[:], T[:])
                    lo = lp.tile([128, PIX], f16, tag=f"l{t}")
                    nc.gpsimd.tensor_sub(lo[:], T[:], hi[:])
                    his.append(hi)
                    los.append(lo)

                # --- blend B_k on PE, stationary-major (4 LDW/image) ---
                pbs = [[pb.tile([128, 512], f32, tag=f"pb{g}{hf}",
                                name=f"pb{g}{hf}")
                        for hf in range(2)] for g in range(2)]
                for si, (stat, srcs, d) in enumerate((
                        (stats[0], his, 0), (stats[0], los, 0),
                        (stats[1], his, 1), (stats[1], los, 1))):
                    for g in range(2):
                        for hf in range(2):
                            hs = slice(hf * 512, (hf + 1) * 512)
                            nc.tensor.matmul(
                                pbs[g][hf][:], stat, srcs[g + d][:, hs],
                                start=(si == 0), stop=(si == 3))
                Bviews = []
                for g in range(2):
                    Bt = bp.tile([128, PIX], bf16, tag=f"B{g}")
                    for hf in range(2):
                        hs = slice(hf * 512, (hf + 1) * 512)
                        nc.vector.tensor_copy(Bt[:, hs], pbs[g][hf][:])
                    Bviews.append(
                        Bt[:].rearrange("p (h w) -> p h w", w=WW))

                # --- main conv, weight-stationary over both banks ---
                pss = [pp.tile([F, BANKN], f32, tag=f"ps{hf}",
                               name=f"ps{hf}")
                       for hf in range(2)]
                for ci in range(21):
                    if ci < 3:
                        di = ci
                        lhsT = wst[:, di * F:(di + 1) * F]
                        rhs = [slv[:, half * 15 + di:half * 15 + di + 15, 0:WO]
                               for half in range(2)]
                    else:
                        g, off = divmod(ci - 3, 9)
                        di, dj = divmod(off, KW)
                        lhsT = wbt[:, (g * 9 + off) * F:(g * 9 + off + 1) * F]
                        rhs = [Bviews[g][:, half * 15 + di:half * 15 + di + 15,
                                         dj:dj + WO]
                               for half in range(2)]
                    for half in range(2):
                        nc.tensor.matmul(
                            pss[half][:], lhsT, rhs[half],
                            start=(ci == 0), stop=(ci == 20))
                for half in range(2):
                    s = (im * 2 + half) * BANKN
                    nc.scalar.activation(
                        out_t[:, s:s + BANKN], pss[half][:], AF.Identity,
                        bias=ct[:, 6:7], scale=1.0)
                    nc.sync.dma_start(y[:, s:s + BANKN], out_t[:, s:s + BANKN])

    nc.compile()
    return nc


def _prep_static(spline_kernel, scale_factor, kan_bias, conv_bias):
    import ml_dtypes

    w6 = (spline_kernel.astype(np.float64)
          * scale_factor.astype(np.float64)[:, None, :]) / 6.0
    w6 = w6.reshape(KH * KW, C, 8, F)
    Wb = np.zeros((18, 128, F), np.float64)
    for off in range(9):
        for g in range(2):
            blk = w6[off][:, 4 * g:4 * g + 4]            # (32c, 4k, F)
            Wb[g * 9 + off] = blk.transpose(1, 0, 2).reshape(128, F)
    wb = np.ascontiguousarray(
        Wb.transpose(1, 0, 2).reshape(128, 18 * F)).astype(ml_dtypes.bfloat16)

    sf9 = scale_factor.astype(np.float64).reshape(9, C, F)
    Ws = np.zeros((3, 96, F), np.float64)
    for di in range(3):
        Ws[di] = sf9[3 * di:3 * di + 3].reshape(96, F)
    ws = np.ascontiguousarray(
        Ws.transpose(1, 0, 2).reshape(96, 3 * F)).astype(ml_dtypes.bfloat16)

    cm = np.array([1.0, -4.0, 6.0, -4.0, 1.0])
    pin = np.arange(128)[:, None]
    pout = np.arange(128)[None, :]
    same_c = (pin % 32) == (pout % 32)
    Ms = []
    for base in (0, 4):
        m = base + pin // 32 - pout // 32
        Ms.append(np.where((m >= 0) & (m <= 4) & same_c,
                           cm[np.clip(m, 0, 4)], 0.0))
    wm = np.ascontiguousarray(
        np.concatenate(Ms, axis=1), np.float32)          # [128, 256]

    consts = np.zeros((128, 8), np.float32)
    p = np.arange(128)
    for t in range(3):
        r = 4 * t + p // 32
        consts[:, t] = -(T0 + HGRID * r) / HGRID          # 5.5 - r
    consts[:, 6] = (kan_bias.astype(np.float64)
                    + conv_bias.astype(np.float64)).astype(np.float32)
    return wb, ws, wm, consts


def kernel(x, spline_kernel, scale_factor, kan_bias, conv_bias):
    from concourse import bass_utils

    x = np.asarray(x, np.float32)
    spline_kernel = np.asarray(spline_kernel, np.float32)
    scale_factor = np.asarray(scale_factor, np.float32)
    kan_bias = np.asarray(kan_bias, np.float32)
    conv_bias = np.asarray(conv_bias, np.float32)

    if "nc" not in _cache:
        _cache["nc"] = _build_program()
    nc = _cache["nc"]

    wb, ws, wm, consts = _prep_static(
        spline_kernel, scale_factor, kan_bias, conv_bias)

    in_maps = []
    for c in range(N_CORES):
        xc = x[c * BPC:(c + 1) * BPC]                     # (4,32,32,32)
        xtc = np.ascontiguousarray(
            xc.transpose(3, 0, 1, 2).reshape(C, BPC * PIX), np.float32
        )
        in_maps.append(
            {"xt": xtc, "wb": wb, "ws": ws, "wm": wm, "consts": consts})

    res = bass_utils.run_bass_kernel_spmd(
        nc, in_maps, core_ids=list(range(N_CORES)),
        **_cache.get("run_kwargs", {})
    )
    _cache["last_result"] = res

    out = np.empty((B, HO, WO, F), np.float32)
    for c in range(N_CORES):
        yc = res.results[c]["y"]                          # (128, 3600)
        out[c * BPC:(c + 1) * BPC] = (
            yc.reshape(F, BPC, HO, WO).transpose(1, 2, 3, 0)
        )
    return out
